# revision 17
# baseline (speedup 1.0000x reference)
"""Trainium2 Bass kernel for nn_COCQCNN_layer (quantum 2x2-patch circuit layer).

Full inputs: x [16, 3, 256, 256] f32, thetas [12] f32, phis [3] f32.
Output: [16, 1, 128, 128] f32 = <Z_0> per 2x2 patch of a 5-qubit circuit.

Algorithm (X-basis form): the 4 per-patch RX gates of a layer are jointly
diagonal in the Hadamard basis of wires 1-4: amplitude (b in {0,1}^4) picks up
phase e^{-i sigma_b}, sigma_b = sum_w +-a_w/2. Complement symmetry
sigma_{~b} = -sigma_b means only 8 of 16 (cos, sin) rows are independent, so
one [128, 512] sigma tile (8 patch-groups x {cos,sin} x 8 b-reps) serves 4096
patches (a "pair" of 2048-patch macros). Fixed per-layer 64x64 real matrices
(thetas/phis only) apply on the TensorEngine as packed 128x128 fp16 matmuls.
Per-patch data enters via cos/sin broadcast matmuls + fp16 elementwise
multiplies (DVE 2x mode; PSUM->SBUF fp16 staging copies split over ACT/DVE).

Sharding: pure data parallel over patches; 8 cores x 8 pairs x 4096 patches.
"""
import sys
import os

sys.path.insert(0, '/opt/trn_rl_repo')

import numpy as np

KAPPA = 2.0 ** -2.5
PI = np.pi
N_CORES = 8
TILES_PER_CORE = 32           # old 1024-patch tiles; 4 per pair
P_TOTAL = 262144              # 16 * 128 * 128
_REPEAT = int(os.environ.get("KERNEL_REPEAT", "1"))
_CACHE = {}


# ----------------------------------------------------------------------------
# host-side constant construction
# ----------------------------------------------------------------------------

def _kron_list(mats):
    out = np.array([[1.0]], np.complex128)
    for m in mats:
        out = np.kron(out, m)
    return out


def _embed(gate2q, wires):
    U = np.zeros((32, 32), np.complex128)
    wc, wt = wires
    for idx_in in range(32):
        bits_in = [(idx_in >> (4 - w)) & 1 for w in range(5)]
        for co in range(2):
            for to in range(2):
                amp = gate2q[co, to, bits_in[wc], bits_in[wt]]
                if amp == 0:
                    continue
                bits_out = list(bits_in)
                bits_out[wc] = co
                bits_out[wt] = to
                idx_out = sum(bits_out[w] << (4 - w) for w in range(5))
                U[idx_out, idx_in] += amp
    return U


def _x_theta(theta):
    e = np.exp(0.5j * theta)
    return np.array([[0, -1j * e], [-1j * np.conj(e), 0]], np.complex128)


def _cu(theta):
    cu = np.zeros((2, 2, 2, 2), np.complex128)
    cu[0, :, 0, :] = np.eye(2)
    cu[1, :, 1, :] = _x_theta(theta)
    return cu


def _cphase(phi):
    g = np.zeros((2, 2, 2, 2), np.complex128)
    g[0, :, 0, :] = np.eye(2)
    g[1, 0, 1, 0] = 1.0
    g[1, 1, 1, 1] = np.exp(1j * phi)
    return g


def _fixed_layer_matrices(thetas, phis):
    H = np.array([[1, 1], [1, -1]], np.complex128) / np.sqrt(2)
    G = _kron_list([np.eye(2), H, H, H, H])
    pairs = [(1, 2), (2, 3), (3, 4), (4, 1)]
    mats = []
    for l in range(3):
        F = np.eye(32, dtype=np.complex128)
        for w in range(4):
            F = _embed(_cu(thetas[4 * l + w]), pairs[w]) @ F
        F = _embed(_cphase(phis[l]), (0, 1)) @ F
        mats.append(G @ F @ G)
    return mats


def _realify(M):
    n = M.shape[0]
    R = np.zeros((2 * n, 2 * n))
    R[0::2, 0::2] = M.real
    R[0::2, 1::2] = -M.imag
    R[1::2, 0::2] = M.imag
    R[1::2, 1::2] = M.real
    return R


def _expand_group(M64):
    """64x64 real on (a,b,r) -> 128x128 on device rows a*64+g*32+b*2+r."""
    F = np.zeros((128, 128))
    ar = np.arange(2)
    comp = ((ar[:, None, None] * 16 + np.arange(16)[None, :, None]) * 2
            + np.arange(2)[None, None, :])
    row = (ar[:, None, None] * 64 + np.arange(16)[None, :, None] * 2
           + np.arange(2)[None, None, :])
    comp = comp.reshape(-1)
    row = row.reshape(-1)
    for g in range(2):
        F[np.ix_(row + g * 32, row + g * 32)] = M64[np.ix_(comp, comp)]
    return F


def _s_w(b, w):
    return 1.0 if ((b >> (3 - w)) & 1) == 0 else -1.0


def _j_of_b(b):
    return b if b < 8 else 15 - b


def _sgn_of_b(b):
    return 1.0 if b < 8 else -1.0


def _build_constants(thetas, phis):
    thetas = np.asarray(thetas, np.float64)
    phis = np.asarray(phis, np.float64)
    Ft = _fixed_layer_matrices(thetas, phis)
    Fhat = [_expand_group(_realify(M)) for M in Ft]

    SWAP = np.zeros((128, 128))
    for a in range(2):
        for g in range(2):
            for b in range(16):
                for r in range(2):
                    SWAP[a * 64 + g * 32 + b * 2 + r,
                         a * 64 + g * 32 + b * 2 + (1 - r)] = 1.0

    # c_sig: [34, 128]; p-tile col = g_new*16 + t*8 + j
    c_sig = np.zeros((34, 128), np.float32)
    for g in range(8):
        for t in range(2):
            for j in range(8):
                col = g * 16 + t * 8 + j
                for w in range(4):
                    c_sig[g * 4 + w, col] = 0.5 * _s_w(j, w)
                if t == 0:
                    c_sig[32, col] = np.float32(PI / 2)

    def bld_T(T):
        B0 = np.zeros((128, 128))
        for a in range(2):
            for g_old in range(2):
                g_new = 2 * T + g_old
                for b in range(16):
                    j, sg = _j_of_b(b), _sgn_of_b(b)
                    srow = a * 64 + g_old * 32 + b * 2
                    B0[srow + 0, g_new * 16 + 0 * 8 + j] = KAPPA
                    B0[srow + 1, g_new * 16 + 1 * 8 + j] = -KAPPA * sg
        return Fhat[0] @ B0

    def bc_T(T):
        M = np.zeros((128, 128))
        for a in range(2):
            for g_old in range(2):
                g_new = 2 * T + g_old
                for b in range(16):
                    j = _j_of_b(b)
                    srow = a * 64 + g_old * 32 + b * 2
                    M[srow + 0, g_new * 16 + 0 * 8 + j] = 1.0
                    M[srow + 1, g_new * 16 + 0 * 8 + j] = 1.0
        return M

    def bs_T(T):
        M = np.zeros((128, 128))
        for a in range(2):
            for g_old in range(2):
                g_new = 2 * T + g_old
                for b in range(16):
                    j, sg = _j_of_b(b), _sgn_of_b(b)
                    srow = a * 64 + g_old * 32 + b * 2
                    M[srow + 0, g_new * 16 + 1 * 8 + j] = -sg
                    M[srow + 1, g_new * 16 + 1 * 8 + j] = +sg
        return M

    # ev via squares: 2*sum(u*v) = 0.5*sum((u+v)^2 - (u-v)^2); P folds u+-v
    # into the last layer's matmuls so ACT Square reads psi3 straight from PSUM.
    P = np.zeros((128, 128))
    P[0:64, 0:64] = np.eye(64)
    P[0:64, 64:128] = np.eye(64)
    P[64:128, 0:64] = np.eye(64)
    P[64:128, 64:128] = -np.eye(64)

    c_ev = np.zeros((4, 128, 8), np.float16)
    for sl in range(4):
        for g in range(2):
            c_ev[sl, g * 32:(g + 1) * 32, 2 * sl + g] = 0.5
            c_ev[sl, 64 + g * 32:64 + (g + 1) * 32, 2 * sl + g] = -0.5

    bld_all = [bld_T(T).T for T in range(4)]
    bc_all = [bc_T(T).T for T in range(4)]
    bs_all = [bs_T(T).T for T in range(4)]
    return dict(
        c_sig=np.ascontiguousarray(c_sig),
        c_bld=np.ascontiguousarray(np.stack(bld_all).astype(np.float16)),
        c_bc=np.ascontiguousarray(np.stack(bc_all).astype(np.float16)),
        c_bs=np.ascontiguousarray(np.stack(bs_all).astype(np.float16)),
        # row-tiling form: per-T weights live in disjoint 32-row strips
        c_bldA=np.ascontiguousarray(sum(bld_all).astype(np.float16)),
        c_bcA=np.ascontiguousarray(sum(bc_all).astype(np.float16)),
        c_bsA=np.ascontiguousarray(sum(bs_all).astype(np.float16)),
        c_f=np.ascontiguousarray(
            np.stack([Fhat[1].T, (Fhat[1] @ SWAP).T,
                      (P @ Fhat[2]).T, (P @ Fhat[2] @ SWAP).T]).astype(np.float16)),
        c_ev=np.ascontiguousarray(c_ev),
    )


def _angle_blocks(pix):
    """pix [P, 12] f32 -> A [P/4096 pairs, 3, 34, 512] f32.
    Patch linear order ((m*2+tt)*2+g)*512+n; pair P holds m in {2P, 2P+1};
    p-tile group g_new = (m%2)*4 + tt*2 + g; row g_new*4+w = angle of wire w,
    row 32 = 1.0 (pi/2 offset selector)."""
    n_pair = pix.shape[0] // 4096
    px = pix.reshape(n_pair, 2, 2, 2, 512, 12)   # (pair, m', tt, g, n, 12)
    A = np.zeros((n_pair, 3, 34, 512), np.float32)
    for l in range(3):
        for mp in range(2):
            for tt in range(2):
                for g in range(2):
                    g_new = 4 * mp + 2 * tt + g
                    for w in range(4):
                        A[:, l, g_new * 4 + w, :] = px[:, mp, tt, g, :, 4 * l + w]
        A[:, l, 32, :] = 1.0
    return A


# ----------------------------------------------------------------------------
# device program
# ----------------------------------------------------------------------------

def _build_nc(n_tiles=TILES_PER_CORE, repeat=1):
    """n_tiles = old 1024-patch tiles per core; must be divisible by 4."""
    import contextlib
    import concourse.mybir as mybir
    from concourse import bacc
    from concourse.tile import TileContext

    F32 = mybir.dt.float32
    F32R = mybir.dt.float32r
    F16 = mybir.dt.float16
    AF = mybir.ActivationFunctionType

    assert n_tiles % 4 == 0
    n_pair = n_tiles // 4

    nc = bacc.Bacc(None, target_bir_lowering=False, debug=False)
    ang_d = nc.declare_dram_parameter("ang", [n_pair, 3, 34, 512], F32R,
                                      isOutput=False)
    csig_d = nc.declare_dram_parameter("c_sig", [34, 128], F32R, isOutput=False)
    cbld_d = nc.declare_dram_parameter("c_bld", [4, 128, 128], F16, isOutput=False)
    cbc_d = nc.declare_dram_parameter("c_bc", [4, 128, 128], F16, isOutput=False)
    cbs_d = nc.declare_dram_parameter("c_bs", [4, 128, 128], F16, isOutput=False)
    cf_d = nc.declare_dram_parameter("c_f", [4, 128, 128], F16, isOutput=False)
    cev_d = nc.declare_dram_parameter("c_ev", [4, 128, 8], F16, isOutput=False)
    cbldA_d = nc.declare_dram_parameter("c_bldA", [128, 128], F16, isOutput=False)
    cbcA_d = nc.declare_dram_parameter("c_bcA", [128, 128], F16, isOutput=False)
    cbsA_d = nc.declare_dram_parameter("c_bsA", [128, 128], F16, isOutput=False)
    ev_d = nc.declare_dram_parameter("ev", [n_pair, 8, 512], F32, isOutput=True)

    BA = int(os.environ.get("BUFS_ANG", "6"))
    BP = int(os.environ.get("BUFS_P", "3"))
    BW = int(os.environ.get("BUFS_WRK", "3"))
    BC = int(os.environ.get("BUFS_CS", "4"))
    BS = int(os.environ.get("BUFS_PSIS", "3"))
    BM = int(os.environ.get("BUFS_MMT", "4"))
    # how many of each 4 consecutive bc/bs staging copies run on DVE (rest ACT)
    NDVE_CS = int(os.environ.get("NDVE_CS", "2"))
    # 1 = run the m2 multiply of every mh-layer on gpsimd instead of DVE
    NGP_MUL = int(os.environ.get("NGP_MUL", "0"))
    DO_WRAP = os.environ.get("NO_WRAP", "0") != "1"
    # 1 = issue bld/bc/bs broadcasts as 4-way row-tiled concurrent matmuls
    ROWTILE = os.environ.get("ROWTILE", "0") == "1"

    with TileContext(nc) as tc:
        with (
            tc.tile_pool(name="const", bufs=1) as cpool,
            tc.tile_pool(name="angp", bufs=BA) as angp,
            tc.tile_pool(name="wrk", bufs=BW) as wrk,
            tc.tile_pool(name="pp", bufs=BP) as pp,
            tc.tile_pool(name="csp", bufs=BC) as csp,
            tc.tile_pool(name="psis", bufs=BS) as psis,
            tc.tile_pool(name="mmt", bufs=BM) as mmt,
            tc.tile_pool(name="qp", bufs=2) as qp,
            tc.tile_pool(name="evs", bufs=2) as evs,
            tc.tile_pool(name="sigp", bufs=1, space="PSUM") as sigp,
            tc.tile_pool(name="bcbs", bufs=2, space="PSUM") as bcbs,
            tc.tile_pool(name="psip", bufs=2, space="PSUM") as psip,
            tc.tile_pool(name="evp", bufs=1, space="PSUM") as evp,
        ):
            c_sig = cpool.tile([34, 128], F32R)
            nc.sync.dma_start(out=c_sig[:], in_=csig_d[:])
            c_bld = []
            c_bc = []
            c_bs = []
            for T in range(4):
                tb = cpool.tile([128, 128], F16, tag=f"bld{T}")
                nc.sync.dma_start(out=tb[:], in_=cbld_d[T])
                c_bld.append(tb)
                t1 = cpool.tile([128, 128], F16, tag=f"bc{T}")
                nc.sync.dma_start(out=t1[:], in_=cbc_d[T])
                c_bc.append(t1)
                t2 = cpool.tile([128, 128], F16, tag=f"bs{T}")
                nc.sync.dma_start(out=t2[:], in_=cbs_d[T])
                c_bs.append(t2)
            c_f = []
            for k in range(4):
                tf = cpool.tile([128, 128], F16, tag=f"f{k}")
                nc.sync.dma_start(out=tf[:], in_=cf_d[k])
                c_f.append(tf)
            if ROWTILE:
                c_bldA = cpool.tile([128, 128], F16, tag="bldA")
                nc.sync.dma_start(out=c_bldA[:], in_=cbldA_d[:])
                c_bcA = cpool.tile([128, 128], F16, tag="bcA")
                nc.sync.dma_start(out=c_bcA[:], in_=cbcA_d[:])
                c_bsA = cpool.tile([128, 128], F16, tag="bsA")
                nc.sync.dma_start(out=c_bsA[:], in_=cbsA_d[:])
            c_ev = []
            for sl in range(4):
                te = cpool.tile([128, 8], F16, tag=f"ev{sl}")
                nc.sync.dma_start(out=te[:], in_=cev_d[sl])
                c_ev.append(te)

            rep_ctx = (tc.For_i(0, repeat, 1) if repeat > 1
                       else contextlib.nullcontext())
            with rep_ctx:
                for Pp in range(n_pair):
                    a_ls = []
                    for l in range(3):
                        a_l = angp.tile([34, 512], F32R, tag=f"ang{l}")
                        nc.gpsimd.dma_start(out=a_l[:], in_=ang_d[Pp, l])
                        a_ls.append(a_l)

                    psi_s = [None, None]      # per mh, fp16 SBUF [128,1024]
                    q2 = [None, None]         # per mh, fp16 squares [128,1024]
                    cs_k = 0                  # bc/bs staging copy round-robin
                    for l in range(3):
                        sig = sigp.tile([128, 512], F32, tag="sig")
                        nc.tensor.matmul(sig[:], c_sig[:], a_ls[l][:],
                                         start=True, stop=True)
                        if DO_WRAP:
                            w = wrk.tile([128, 512], F32, tag="w")
                            nc.vector.add_range_wrap(
                                w[:], sig[:], shift=0.0, bound=PI, period=2 * PI)
                            p = pp.tile([128, 512], F16, tag="p")
                            nc.scalar.activation(p[:], w[:], AF.Sin)
                        else:
                            p = pp.tile([128, 512], F16, tag="p")
                            nc.scalar.activation(p[:], sig[:], AF.Sin)

                        bc_sv = [None, None]
                        bs_sv = [None, None]
                        if ROWTILE and l > 0:
                            # 4-way row-tiled bursts: each T's weights live in
                            # strip 32T of the combined matrix; 4 concurrent MMs
                            for W, outs in ((c_bcA, bc_sv), (c_bsA, bs_sv)):
                                xP = [bcbs.tile([128, 1024], F32, tag="bcbs",
                                                name=f"xP{mh}")
                                      for mh in range(2)]
                                for T in range(4):
                                    s32 = slice(32 * T, 32 * T + 32)
                                    sc = slice(512 * (T % 2), 512 * (T % 2) + 512)
                                    nc.tensor.matmul(xP[T // 2][:, sc],
                                                     W[s32, :], p[s32, :],
                                                     start=True, stop=True,
                                                     tile_position=(32 * T, 0))
                                for mh in range(2):
                                    dst = csp.tile([128, 1024], F16, tag="cs")
                                    if cs_k % 4 < NDVE_CS:
                                        nc.vector.tensor_copy(dst[:], xP[mh][:])
                                    else:
                                        nc.scalar.copy(out=dst[:], in_=xP[mh][:])
                                    cs_k += 1
                                    outs[mh] = dst

                        for mh in range(2):
                            T0 = 2 * mh
                            psi_p = [psip.tile([128, 512], F32, tag="psi",
                                               name=f"psi{ti}")
                                     for ti in range(2)]
                            if l == 0:
                                for ti, T in enumerate((T0, T0 + 1)):
                                    if ROWTILE:
                                        s32 = slice(32 * T, 32 * T + 32)
                                        nc.tensor.matmul(
                                            psi_p[ti][:], c_bldA[s32, :],
                                            p[s32, :], start=True, stop=True,
                                            tile_position=(32 * T, 0))
                                    else:
                                        nc.tensor.matmul(psi_p[ti][:],
                                                         c_bld[T][:], p[:],
                                                         start=True, stop=True)
                            else:
                                if ROWTILE:
                                    bc_s = bc_sv[mh]
                                    bs_s = bs_sv[mh]
                                else:
                                    bcP = bcbs.tile([128, 1024], F32, tag="bcbs")
                                    nc.tensor.matmul(bcP[:, 0:512], c_bc[T0][:],
                                                     p[:], start=True, stop=True)
                                    nc.tensor.matmul(bcP[:, 512:1024],
                                                     c_bc[T0 + 1][:],
                                                     p[:], start=True, stop=True)
                                    bsP = bcbs.tile([128, 1024], F32, tag="bcbs")
                                    nc.tensor.matmul(bsP[:, 0:512], c_bs[T0][:],
                                                     p[:], start=True, stop=True)
                                    nc.tensor.matmul(bsP[:, 512:1024],
                                                     c_bs[T0 + 1][:],
                                                     p[:], start=True, stop=True)
                                    bc_s = csp.tile([128, 1024], F16, tag="cs")
                                    bs_s = csp.tile([128, 1024], F16, tag="cs")
                                    for src, dst in ((bcP, bc_s), (bsP, bs_s)):
                                        if cs_k % 4 < NDVE_CS:
                                            nc.vector.tensor_copy(dst[:], src[:])
                                        else:
                                            nc.scalar.copy(out=dst[:],
                                                           in_=src[:])
                                        cs_k += 1
                                m1 = mmt.tile([128, 1024], F16, tag="m")
                                nc.vector.tensor_mul(m1[:], bc_s[:], psi_s[mh][:])
                                m2 = mmt.tile([128, 1024], F16, tag="m")
                                if NGP_MUL:
                                    nc.gpsimd.tensor_mul(m2[:], bs_s[:],
                                                         psi_s[mh][:])
                                else:
                                    nc.vector.tensor_mul(m2[:], bs_s[:],
                                                         psi_s[mh][:])
                                base = 2 * (l - 1)
                                for ti in range(2):
                                    sl_c = slice(512 * ti, 512 * ti + 512)
                                    nc.tensor.matmul(psi_p[ti][:], c_f[base][:],
                                                     m1[:, sl_c], start=True,
                                                     stop=False)
                                    nc.tensor.matmul(psi_p[ti][:],
                                                     c_f[base + 1][:],
                                                     m2[:, sl_c], start=False,
                                                     stop=True)
                            if l < 2:
                                ps_t = psis.tile([128, 1024], F16, tag="psis")
                                for ti in range(2):
                                    nc.scalar.copy(
                                        out=ps_t[:, 512 * ti:512 * ti + 512],
                                        in_=psi_p[ti][:])
                                psi_s[mh] = ps_t
                            else:
                                q_t = qp.tile([128, 1024], F16, tag="q2")
                                for ti in range(2):
                                    nc.scalar.activation(
                                        q_t[:, 512 * ti:512 * ti + 512],
                                        psi_p[ti][:], AF.Square)
                                q2[mh] = q_t

                    evt = evp.tile([8, 512], F32, tag="ev")
                    for mh in range(2):
                        for ti in range(2):
                            sl = 2 * mh + ti
                            nc.tensor.matmul(evt[:],
                                             c_ev[sl][:],
                                             q2[mh][:, 512 * ti:512 * ti + 512],
                                             start=(sl == 0), stop=(sl == 3))
                    ev_s = evs.tile([8, 512], F32, tag="evs")
                    nc.scalar.copy(out=ev_s[:], in_=evt[:])
                    nc.sync.dma_start(out=ev_d[Pp], in_=ev_s[:])

    nc.finalize()
    return nc


def _get_nc(repeat=_REPEAT):
    key = ("nc", repeat)
    if key not in _CACHE:
        _CACHE[key] = _build_nc(repeat=repeat)
    return _CACHE[key]


# ----------------------------------------------------------------------------
# entry point
# ----------------------------------------------------------------------------

def kernel(x, thetas, phis):
    from concourse.bass_utils import run_bass_kernel_spmd

    x = np.asarray(x, np.float32)
    thetas = np.asarray(thetas, np.float32)
    phis = np.asarray(phis, np.float32)
    B, C, H, W = x.shape
    H2, W2 = H // 2, W // 2
    pix = (x.reshape(B, 3, H2, 2, W2, 2)
             .transpose(0, 2, 4, 1, 3, 5)
             .reshape(B * H2 * W2, 12))

    A = _angle_blocks(pix)                    # [64 pairs, 3, 34, 512]
    consts = _build_constants(thetas, phis)
    per_core = A.shape[0] // N_CORES
    in_maps = [{"ang": np.ascontiguousarray(A[c * per_core:(c + 1) * per_core]),
                **consts} for c in range(N_CORES)]

    nc = _get_nc()
    res = run_bass_kernel_spmd(nc, in_maps, list(range(N_CORES)))
    # ev_d [n_pair, 8, 512]: row 2*sl+g of pair P -> old tile 4*P+sl
    evs = [res.results[c]["ev"].reshape(-1, 4, 2, 512).reshape(-1)
           for c in range(N_CORES)]
    ev = np.concatenate(evs)
    return ev.reshape(B, 1, H2, W2).astype(np.float32)


# revision 28
# speedup vs baseline: 2.1699x; 2.1699x over previous
"""Trainium2 Bass kernel for nn_COCQCNN_layer (quantum 2x2-patch circuit layer).

Full inputs: x [16, 3, 256, 256] f32, thetas [12] f32, phis [3] f32.
Output: [16, 1, 128, 128] f32 = <Z_0> per 2x2 patch of a 5-qubit circuit.

Algorithm (X-basis form): the 4 per-patch RX gates of a layer are jointly
diagonal in the Hadamard basis of wires 1-4: amplitude (b in {0,1}^4) picks up
phase e^{-i sigma_b}, sigma_b = sum_w +-a_w/2. Complement symmetry
sigma_{~b} = -sigma_b means only 8 of 16 (cos, sin) rows are independent, so
one [128, 512] sigma tile (8 patch-groups x {cos,sin} x 8 b-reps) serves 4096
patches (a "pair" of 2048-patch macros). Fixed per-layer 64x64 real matrices
(thetas/phis only) apply on the TensorEngine as packed 128x128 fp16 matmuls.
Per-patch data enters via cos/sin broadcast matmuls + fp16 elementwise
multiplies (DVE 2x mode; PSUM->SBUF fp16 staging copies split over ACT/DVE).

Sharding: pure data parallel over patches; 8 cores x 8 pairs x 4096 patches.
"""
import sys
import os

sys.path.insert(0, '/opt/trn_rl_repo')

import numpy as np

KAPPA = 2.0 ** -2.5
PI = np.pi
N_CORES = 8
TILES_PER_CORE = 32           # old 1024-patch tiles; 4 per pair
P_TOTAL = 262144              # 16 * 128 * 128
_REPEAT = int(os.environ.get("KERNEL_REPEAT", "1"))
_CACHE = {}


# ----------------------------------------------------------------------------
# host-side constant construction
# ----------------------------------------------------------------------------

def _kron_list(mats):
    out = np.array([[1.0]], np.complex128)
    for m in mats:
        out = np.kron(out, m)
    return out


def _embed(gate2q, wires):
    U = np.zeros((32, 32), np.complex128)
    wc, wt = wires
    for idx_in in range(32):
        bits_in = [(idx_in >> (4 - w)) & 1 for w in range(5)]
        for co in range(2):
            for to in range(2):
                amp = gate2q[co, to, bits_in[wc], bits_in[wt]]
                if amp == 0:
                    continue
                bits_out = list(bits_in)
                bits_out[wc] = co
                bits_out[wt] = to
                idx_out = sum(bits_out[w] << (4 - w) for w in range(5))
                U[idx_out, idx_in] += amp
    return U


def _x_theta(theta):
    e = np.exp(0.5j * theta)
    return np.array([[0, -1j * e], [-1j * np.conj(e), 0]], np.complex128)


def _cu(theta):
    cu = np.zeros((2, 2, 2, 2), np.complex128)
    cu[0, :, 0, :] = np.eye(2)
    cu[1, :, 1, :] = _x_theta(theta)
    return cu


def _cphase(phi):
    g = np.zeros((2, 2, 2, 2), np.complex128)
    g[0, :, 0, :] = np.eye(2)
    g[1, 0, 1, 0] = 1.0
    g[1, 1, 1, 1] = np.exp(1j * phi)
    return g


def _fixed_layer_matrices(thetas, phis):
    H = np.array([[1, 1], [1, -1]], np.complex128) / np.sqrt(2)
    G = _kron_list([np.eye(2), H, H, H, H])
    pairs = [(1, 2), (2, 3), (3, 4), (4, 1)]
    mats = []
    for l in range(3):
        F = np.eye(32, dtype=np.complex128)
        for w in range(4):
            F = _embed(_cu(thetas[4 * l + w]), pairs[w]) @ F
        F = _embed(_cphase(phis[l]), (0, 1)) @ F
        mats.append(G @ F @ G)
    return mats


def _realify(M):
    n = M.shape[0]
    R = np.zeros((2 * n, 2 * n))
    R[0::2, 0::2] = M.real
    R[0::2, 1::2] = -M.imag
    R[1::2, 0::2] = M.imag
    R[1::2, 1::2] = M.real
    return R


def _expand_group(M64):
    """64x64 real on (a,b,r) -> 128x128 on device rows a*64+g*32+b*2+r."""
    F = np.zeros((128, 128))
    ar = np.arange(2)
    comp = ((ar[:, None, None] * 16 + np.arange(16)[None, :, None]) * 2
            + np.arange(2)[None, None, :])
    row = (ar[:, None, None] * 64 + np.arange(16)[None, :, None] * 2
           + np.arange(2)[None, None, :])
    comp = comp.reshape(-1)
    row = row.reshape(-1)
    for g in range(2):
        F[np.ix_(row + g * 32, row + g * 32)] = M64[np.ix_(comp, comp)]
    return F


def _s_w(b, w):
    return 1.0 if ((b >> (3 - w)) & 1) == 0 else -1.0


def _j_of_b(b):
    return b if b < 8 else 15 - b


def _sgn_of_b(b):
    return 1.0 if b < 8 else -1.0


def _build_constants(thetas, phis):
    thetas = np.asarray(thetas, np.float64)
    phis = np.asarray(phis, np.float64)
    Ft = _fixed_layer_matrices(thetas, phis)
    Fhat = [_expand_group(_realify(M)) for M in Ft]

    SWAP = np.zeros((128, 128))
    for a in range(2):
        for g in range(2):
            for b in range(16):
                for r in range(2):
                    SWAP[a * 64 + g * 32 + b * 2 + r,
                         a * 64 + g * 32 + b * 2 + (1 - r)] = 1.0

    # c_sig: [34, 128]; p-tile col = g_new*16 + t*8 + j
    c_sig = np.zeros((34, 128), np.float32)
    for g in range(8):
        for t in range(2):
            for j in range(8):
                col = g * 16 + t * 8 + j
                for w in range(4):
                    c_sig[g * 4 + w, col] = 0.5 * _s_w(j, w)
                if t == 0:
                    c_sig[32, col] = np.float32(PI / 2)

    def bld_T(T):
        B0 = np.zeros((128, 128))
        for a in range(2):
            for g_old in range(2):
                g_new = 2 * T + g_old
                for b in range(16):
                    j, sg = _j_of_b(b), _sgn_of_b(b)
                    srow = a * 64 + g_old * 32 + b * 2
                    B0[srow + 0, g_new * 16 + 0 * 8 + j] = KAPPA
                    B0[srow + 1, g_new * 16 + 1 * 8 + j] = -KAPPA * sg
        return Fhat[0] @ B0

    def bc_T(T):
        M = np.zeros((128, 128))
        for a in range(2):
            for g_old in range(2):
                g_new = 2 * T + g_old
                for b in range(16):
                    j = _j_of_b(b)
                    srow = a * 64 + g_old * 32 + b * 2
                    M[srow + 0, g_new * 16 + 0 * 8 + j] = 1.0
                    M[srow + 1, g_new * 16 + 0 * 8 + j] = 1.0
        return M

    def bs_T(T):
        M = np.zeros((128, 128))
        for a in range(2):
            for g_old in range(2):
                g_new = 2 * T + g_old
                for b in range(16):
                    j, sg = _j_of_b(b), _sgn_of_b(b)
                    srow = a * 64 + g_old * 32 + b * 2
                    M[srow + 0, g_new * 16 + 1 * 8 + j] = -sg
                    M[srow + 1, g_new * 16 + 1 * 8 + j] = +sg
        return M

    # ev via squares: 2*sum(u*v) = 0.5*sum((u+v)^2 - (u-v)^2); P folds u+-v
    # into the last layer's matmuls so ACT Square reads psi3 straight from PSUM.
    P = np.zeros((128, 128))
    P[0:64, 0:64] = np.eye(64)
    P[0:64, 64:128] = np.eye(64)
    P[64:128, 0:64] = np.eye(64)
    P[64:128, 64:128] = -np.eye(64)

    c_ev = np.zeros((4, 128, 8), np.float16)
    for sl in range(4):
        for g in range(2):
            c_ev[sl, g * 32:(g + 1) * 32, 2 * sl + g] = 0.5
            c_ev[sl, 64 + g * 32:64 + (g + 1) * 32, 2 * sl + g] = -0.5

    bld_all = [bld_T(T).T for T in range(4)]
    bc_all = [bc_T(T).T for T in range(4)]
    bs_all = [bs_T(T).T for T in range(4)]
    return dict(
        c_sig=np.ascontiguousarray(c_sig),
        c_bld=np.ascontiguousarray(np.stack(bld_all).astype(np.float16)),
        c_bc=np.ascontiguousarray(np.stack(bc_all).astype(np.float16)),
        c_bs=np.ascontiguousarray(np.stack(bs_all).astype(np.float16)),
        c_f=np.ascontiguousarray(
            np.stack([Fhat[1].T, (Fhat[1] @ SWAP).T,
                      (P @ Fhat[2]).T, (P @ Fhat[2] @ SWAP).T]).astype(np.float16)),
        c_ev=np.ascontiguousarray(c_ev),
    )


def _angle_blocks(pix):
    """pix [P, 12] f32 -> A [P/4096 pairs, 3, 34, 512] f32.
    Patch linear order ((m*2+tt)*2+g)*512+n; pair P holds m in {2P, 2P+1};
    p-tile group g_new = (m%2)*4 + tt*2 + g; row g_new*4+w = angle of wire w,
    row 32 = 1.0 (pi/2 offset selector)."""
    n_pair = pix.shape[0] // 4096
    px = pix.reshape(n_pair, 2, 2, 2, 512, 12)   # (pair, m', tt, g, n, 12)
    A = np.zeros((n_pair, 3, 34, 512), np.float32)
    for l in range(3):
        for mp in range(2):
            for tt in range(2):
                for g in range(2):
                    g_new = 4 * mp + 2 * tt + g
                    for w in range(4):
                        A[:, l, g_new * 4 + w, :] = px[:, mp, tt, g, :, 4 * l + w]
        A[:, l, 32, :] = 1.0
    return A


# ----------------------------------------------------------------------------
# device program
# ----------------------------------------------------------------------------

def _build_nc(n_tiles=TILES_PER_CORE, repeat=1):
    """n_tiles = old 1024-patch tiles per core; must be divisible by 4."""
    import contextlib
    import concourse.mybir as mybir
    from concourse import bacc
    from concourse.ap import AP
    from concourse.tile import TileContext

    F32 = mybir.dt.float32
    F32R = mybir.dt.float32r
    F16 = mybir.dt.float16
    AF = mybir.ActivationFunctionType

    assert n_tiles % 4 == 0
    n_pair = n_tiles // 4

    nc = bacc.Bacc(None, target_bir_lowering=False, debug=False)
    ang_d = nc.declare_dram_parameter("ang", [n_pair, 3, 34, 512], F32R,
                                      isOutput=False)
    csig_d = nc.declare_dram_parameter("c_sig", [34, 128], F32R, isOutput=False)
    cbld_d = nc.declare_dram_parameter("c_bld", [4, 128, 128], F16, isOutput=False)
    cbc_d = nc.declare_dram_parameter("c_bc", [4, 128, 128], F16, isOutput=False)
    cbs_d = nc.declare_dram_parameter("c_bs", [4, 128, 128], F16, isOutput=False)
    cf_d = nc.declare_dram_parameter("c_f", [4, 128, 128], F16, isOutput=False)
    cev_d = nc.declare_dram_parameter("c_ev", [4, 128, 8], F16, isOutput=False)
    ev_d = nc.declare_dram_parameter("ev", [n_pair, 8, 512], F32, isOutput=True)

    BA = int(os.environ.get("BUFS_ANG", "9"))
    BP = int(os.environ.get("BUFS_P", "5"))
    BW = int(os.environ.get("BUFS_WRK", "5"))
    BC = int(os.environ.get("BUFS_CS", "6"))
    BS = int(os.environ.get("BUFS_PSIS", "4"))
    BM = int(os.environ.get("BUFS_MMT", "6"))
    # how many of each 4 consecutive bc/bs staging copies run on DVE (rest ACT)
    NDVE_CS = int(os.environ.get("NDVE_CS", "2"))
    # of each mh-layer's 2 psi staging copies, how many run on DVE (rest ACT)
    NDVE_PSI = int(os.environ.get("NDVE_PSI", "0"))
    DO_WRAP = os.environ.get("NO_WRAP", "0") != "1"

    with TileContext(nc) as tc:
        with (
            tc.tile_pool(name="const", bufs=1) as cpool,
            tc.tile_pool(name="angp", bufs=BA) as angp,
            tc.tile_pool(name="wrk", bufs=BW) as wrk,
            tc.tile_pool(name="pp", bufs=BP) as pp,
            tc.tile_pool(name="csp", bufs=BC) as csp,
            tc.tile_pool(name="psis", bufs=BS) as psis,
            tc.tile_pool(name="mmt", bufs=BM) as mmt,
            tc.tile_pool(name="qp", bufs=2) as qp,
            tc.tile_pool(name="evs", bufs=2) as evs,
            tc.tile_pool(name="sigp", bufs=1, space="PSUM") as sigp,
            tc.tile_pool(name="bcbs", bufs=2, space="PSUM") as bcbs,
            tc.tile_pool(name="psip", bufs=2, space="PSUM") as psip,
            tc.tile_pool(name="evp", bufs=1, space="PSUM") as evp,
        ):
            c_sig = cpool.tile([34, 128], F32R)
            nc.sync.dma_start(out=c_sig[:], in_=csig_d[:])
            c_bld = []
            c_bc = []
            c_bs = []
            for T in range(4):
                tb = cpool.tile([128, 128], F16, tag=f"bld{T}")
                nc.sync.dma_start(out=tb[:], in_=cbld_d[T])
                c_bld.append(tb)
                t1 = cpool.tile([128, 128], F16, tag=f"bc{T}")
                nc.sync.dma_start(out=t1[:], in_=cbc_d[T])
                c_bc.append(t1)
                t2 = cpool.tile([128, 128], F16, tag=f"bs{T}")
                nc.sync.dma_start(out=t2[:], in_=cbs_d[T])
                c_bs.append(t2)
            c_f = []
            for k in range(4):
                tf = cpool.tile([128, 128], F16, tag=f"f{k}")
                nc.sync.dma_start(out=tf[:], in_=cf_d[k])
                c_f.append(tf)
            c_ev = []
            for sl in range(4):
                te = cpool.tile([128, 8], F16, tag=f"ev{sl}")
                nc.sync.dma_start(out=te[:], in_=cev_d[sl])
                c_ev.append(te)

            rep_ctx = (tc.For_i(0, repeat, 1) if repeat > 1
                       else contextlib.nullcontext())
            with rep_ctx:
                for Pp in range(n_pair):
                    a_ls = []
                    for l in range(3):
                        a_l = angp.tile([34, 512], F32R, tag=f"ang{l}")
                        nc.gpsimd.dma_start(out=a_l[:], in_=ang_d[Pp, l])
                        a_ls.append(a_l)

                    psi_s = [None, None]      # per mh, fp16 SBUF [128,1024]
                    q2 = [None, None]         # per mh, fp16 squares [128,1024]
                    cs_k = 0                  # bc/bs staging copy round-robin
                    for l in range(3):
                        sig = sigp.tile([128, 512], F32, tag="sig")
                        nc.tensor.matmul(sig[:], c_sig[:], a_ls[l][:],
                                         start=True, stop=True)
                        if DO_WRAP:
                            w = wrk.tile([128, 512], F32, tag="w")
                            nc.vector.add_range_wrap(
                                w[:], sig[:], shift=0.0, bound=PI, period=2 * PI)
                            p = pp.tile([128, 512], F16, tag="p")
                            nc.scalar.activation(p[:], w[:], AF.Sin)
                        else:
                            p = pp.tile([128, 512], F16, tag="p")
                            nc.scalar.activation(p[:], sig[:], AF.Sin)

                        for mh in range(2):
                            T0 = 2 * mh
                            psi_p = [psip.tile([128, 512], F32, tag="psi",
                                               name=f"psi{ti}")
                                     for ti in range(2)]
                            if l == 0:
                                for ti, T in enumerate((T0, T0 + 1)):
                                    nc.tensor.matmul(psi_p[ti][:],
                                                     c_bld[T][:], p[:],
                                                     start=True, stop=True)
                            else:
                                bcP = bcbs.tile([128, 1024], F32, tag="bcbs")
                                nc.tensor.matmul(bcP[:, 0:512], c_bc[T0][:],
                                                 p[:], start=True, stop=True)
                                nc.tensor.matmul(bcP[:, 512:1024],
                                                 c_bc[T0 + 1][:],
                                                 p[:], start=True, stop=True)
                                bsP = bcbs.tile([128, 1024], F32, tag="bcbs")
                                nc.tensor.matmul(bsP[:, 0:512], c_bs[T0][:],
                                                 p[:], start=True, stop=True)
                                nc.tensor.matmul(bsP[:, 512:1024],
                                                 c_bs[T0 + 1][:],
                                                 p[:], start=True, stop=True)
                                bc_s = csp.tile([128, 1024], F16, tag="cs")
                                bs_s = csp.tile([128, 1024], F16, tag="cs")
                                for src_t, dst in ((bcP, bc_s), (bsP, bs_s)):
                                    if cs_k % 4 < NDVE_CS:
                                        nc.vector.tensor_copy(dst[:], src_t[:])
                                    else:
                                        nc.scalar.copy(out=dst[:], in_=src_t[:])
                                    cs_k += 1
                                m1 = mmt.tile([128, 1024], F16, tag="m")
                                nc.vector.tensor_mul(m1[:], bc_s[:], psi_s[mh][:])
                                m2 = mmt.tile([128, 1024], F16, tag="m")
                                nc.vector.tensor_mul(m2[:], bs_s[:],
                                                     psi_s[mh][:])
                                base = 2 * (l - 1)
                                for ti in range(2):
                                    sl_c = slice(512 * ti, 512 * ti + 512)
                                    nc.tensor.matmul(psi_p[ti][:], c_f[base][:],
                                                     m1[:, sl_c], start=True,
                                                     stop=False)
                                    nc.tensor.matmul(psi_p[ti][:],
                                                     c_f[base + 1][:],
                                                     m2[:, sl_c], start=False,
                                                     stop=True)
                            if l < 2:
                                ps_t = psis.tile([128, 1024], F16, tag="psis")
                                for ti in range(2):
                                    if ti < NDVE_PSI:
                                        nc.vector.tensor_copy(
                                            ps_t[:, 512 * ti:512 * ti + 512],
                                            psi_p[ti][:])
                                    else:
                                        nc.scalar.copy(
                                            out=ps_t[:, 512 * ti:512 * ti + 512],
                                            in_=psi_p[ti][:])
                                psi_s[mh] = ps_t
                            else:
                                q_t = qp.tile([128, 1024], F16, tag="q2")
                                for ti in range(2):
                                    nc.scalar.activation(
                                        q_t[:, 512 * ti:512 * ti + 512],
                                        psi_p[ti][:], AF.Square)
                                q2[mh] = q_t

                    evt = evp.tile([8, 512], F32, tag="ev")
                    for mh in range(2):
                        for ti in range(2):
                            sl = 2 * mh + ti
                            nc.tensor.matmul(evt[:],
                                             c_ev[sl][:],
                                             q2[mh][:, 512 * ti:512 * ti + 512],
                                             start=(sl == 0), stop=(sl == 3))
                    ev_s = evs.tile([8, 512], F32, tag="evs")
                    nc.scalar.copy(out=ev_s[:], in_=evt[:])
                    nc.sync.dma_start(out=ev_d[Pp], in_=ev_s[:])

    nc.finalize()
    return nc


def _get_nc(repeat=_REPEAT):
    key = ("nc", repeat)
    if key not in _CACHE:
        _CACHE[key] = _build_nc(repeat=repeat)
    return _CACHE[key]


# ----------------------------------------------------------------------------
# entry point
# ----------------------------------------------------------------------------

def kernel(x, thetas, phis):
    from concourse.bass_utils import run_bass_kernel_spmd

    x = np.asarray(x, np.float32)
    thetas = np.asarray(thetas, np.float32)
    phis = np.asarray(phis, np.float32)
    B, C, H, W = x.shape
    H2, W2 = H // 2, W // 2
    pix = (x.reshape(B, 3, H2, 2, W2, 2)
             .transpose(0, 2, 4, 1, 3, 5)
             .reshape(B * H2 * W2, 12))

    A = _angle_blocks(pix)                    # [64 pairs, 3, 34, 512]
    consts = _build_constants(thetas, phis)
    per_core = A.shape[0] // N_CORES
    in_maps = [{"ang": np.ascontiguousarray(A[c * per_core:(c + 1) * per_core]),
                **consts} for c in range(N_CORES)]

    nc = _get_nc()
    res = run_bass_kernel_spmd(nc, in_maps, list(range(N_CORES)))
    # ev_d [n_pair, 8, 512]: row 2*sl+g of pair P -> old tile 4*P+sl
    evs = [res.results[c]["ev"].reshape(-1, 4, 2, 512).reshape(-1)
           for c in range(N_CORES)]
    ev = np.concatenate(evs)
    return ev.reshape(B, 1, H2, W2).astype(np.float32)


# revision 29
# speedup vs baseline: 3.7158x; 1.7124x over previous
"""Trainium2 Bass kernel for nn_COCQCNN_layer (quantum 2x2-patch circuit layer).

Full inputs: x [16, 3, 256, 256] f32, thetas [12] f32, phis [3] f32.
Output: [16, 1, 128, 128] f32 = <Z_0> per 2x2 patch of a 5-qubit circuit.

Algorithm (X-basis form): the 4 per-patch RX gates of a layer are jointly
diagonal in the Hadamard basis of wires 1-4: amplitude (b in {0,1}^4) picks up
phase e^{-i sigma_b}, sigma_b = sum_w +-a_w/2. Complement symmetry
sigma_{~b} = -sigma_b means only 8 of 16 (cos, sin) rows are independent, so
one [128, 512] sigma tile (8 patch-groups x {cos,sin} x 8 b-reps) serves 4096
patches (a "pair" of 2048-patch macros). Fixed per-layer 64x64 real matrices
(thetas/phis only) apply on the TensorEngine as packed 128x128 fp16 matmuls.
Per-patch data enters via cos/sin broadcast matmuls + fp16 elementwise
multiplies (DVE 2x mode; PSUM->SBUF fp16 staging copies split over ACT/DVE).

Sharding: pure data parallel over patches; 8 cores x 8 pairs x 4096 patches.
"""
import sys
import os

sys.path.insert(0, '/opt/trn_rl_repo')

import numpy as np

KAPPA = 2.0 ** -2.5
PI = np.pi
N_CORES = 8
TILES_PER_CORE = 32           # old 1024-patch tiles; 4 per pair
P_TOTAL = 262144              # 16 * 128 * 128
_REPEAT = int(os.environ.get("KERNEL_REPEAT", "1"))
_CACHE = {}


# ----------------------------------------------------------------------------
# host-side constant construction
# ----------------------------------------------------------------------------

def _kron_list(mats):
    out = np.array([[1.0]], np.complex128)
    for m in mats:
        out = np.kron(out, m)
    return out


def _embed(gate2q, wires):
    U = np.zeros((32, 32), np.complex128)
    wc, wt = wires
    for idx_in in range(32):
        bits_in = [(idx_in >> (4 - w)) & 1 for w in range(5)]
        for co in range(2):
            for to in range(2):
                amp = gate2q[co, to, bits_in[wc], bits_in[wt]]
                if amp == 0:
                    continue
                bits_out = list(bits_in)
                bits_out[wc] = co
                bits_out[wt] = to
                idx_out = sum(bits_out[w] << (4 - w) for w in range(5))
                U[idx_out, idx_in] += amp
    return U


def _x_theta(theta):
    e = np.exp(0.5j * theta)
    return np.array([[0, -1j * e], [-1j * np.conj(e), 0]], np.complex128)


def _cu(theta):
    cu = np.zeros((2, 2, 2, 2), np.complex128)
    cu[0, :, 0, :] = np.eye(2)
    cu[1, :, 1, :] = _x_theta(theta)
    return cu


def _cphase(phi):
    g = np.zeros((2, 2, 2, 2), np.complex128)
    g[0, :, 0, :] = np.eye(2)
    g[1, 0, 1, 0] = 1.0
    g[1, 1, 1, 1] = np.exp(1j * phi)
    return g


def _fixed_layer_matrices(thetas, phis):
    H = np.array([[1, 1], [1, -1]], np.complex128) / np.sqrt(2)
    G = _kron_list([np.eye(2), H, H, H, H])
    pairs = [(1, 2), (2, 3), (3, 4), (4, 1)]
    mats = []
    for l in range(3):
        F = np.eye(32, dtype=np.complex128)
        for w in range(4):
            F = _embed(_cu(thetas[4 * l + w]), pairs[w]) @ F
        F = _embed(_cphase(phis[l]), (0, 1)) @ F
        mats.append(G @ F @ G)
    return mats


def _realify(M):
    n = M.shape[0]
    R = np.zeros((2 * n, 2 * n))
    R[0::2, 0::2] = M.real
    R[0::2, 1::2] = -M.imag
    R[1::2, 0::2] = M.imag
    R[1::2, 1::2] = M.real
    return R


def _expand_group(M64):
    """64x64 real on (a,b,r) -> 128x128 on device rows a*64+g*32+b*2+r."""
    F = np.zeros((128, 128))
    ar = np.arange(2)
    comp = ((ar[:, None, None] * 16 + np.arange(16)[None, :, None]) * 2
            + np.arange(2)[None, None, :])
    row = (ar[:, None, None] * 64 + np.arange(16)[None, :, None] * 2
           + np.arange(2)[None, None, :])
    comp = comp.reshape(-1)
    row = row.reshape(-1)
    for g in range(2):
        F[np.ix_(row + g * 32, row + g * 32)] = M64[np.ix_(comp, comp)]
    return F


def _s_w(b, w):
    return 1.0 if ((b >> (3 - w)) & 1) == 0 else -1.0


def _j_of_b(b):
    return b if b < 8 else 15 - b


def _sgn_of_b(b):
    return 1.0 if b < 8 else -1.0


def _build_constants(thetas, phis):
    thetas = np.asarray(thetas, np.float64)
    phis = np.asarray(phis, np.float64)
    Ft = _fixed_layer_matrices(thetas, phis)
    Fhat = [_expand_group(_realify(M)) for M in Ft]

    SWAP = np.zeros((128, 128))
    for a in range(2):
        for g in range(2):
            for b in range(16):
                for r in range(2):
                    SWAP[a * 64 + g * 32 + b * 2 + r,
                         a * 64 + g * 32 + b * 2 + (1 - r)] = 1.0

    # c_sig: [34, 128]; p-tile col = g_new*16 + t*8 + j
    c_sig = np.zeros((34, 128), np.float32)
    for g in range(8):
        for t in range(2):
            for j in range(8):
                col = g * 16 + t * 8 + j
                for w in range(4):
                    c_sig[g * 4 + w, col] = 0.5 * _s_w(j, w)
                if t == 0:
                    c_sig[32, col] = np.float32(PI / 2)

    def bld_T(T):
        B0 = np.zeros((128, 128))
        for a in range(2):
            for g_old in range(2):
                g_new = 2 * T + g_old
                for b in range(16):
                    j, sg = _j_of_b(b), _sgn_of_b(b)
                    srow = a * 64 + g_old * 32 + b * 2
                    B0[srow + 0, g_new * 16 + 0 * 8 + j] = KAPPA
                    B0[srow + 1, g_new * 16 + 1 * 8 + j] = -KAPPA * sg
        return Fhat[0] @ B0

    def bc_T(T):
        M = np.zeros((128, 128))
        for a in range(2):
            for g_old in range(2):
                g_new = 2 * T + g_old
                for b in range(16):
                    j = _j_of_b(b)
                    srow = a * 64 + g_old * 32 + b * 2
                    M[srow + 0, g_new * 16 + 0 * 8 + j] = 1.0
                    M[srow + 1, g_new * 16 + 0 * 8 + j] = 1.0
        return M

    def bs_T(T):
        M = np.zeros((128, 128))
        for a in range(2):
            for g_old in range(2):
                g_new = 2 * T + g_old
                for b in range(16):
                    j, sg = _j_of_b(b), _sgn_of_b(b)
                    srow = a * 64 + g_old * 32 + b * 2
                    M[srow + 0, g_new * 16 + 1 * 8 + j] = -sg
                    M[srow + 1, g_new * 16 + 1 * 8 + j] = +sg
        return M

    # ev via squares: 2*sum(u*v) = 0.5*sum((u+v)^2 - (u-v)^2); P folds u+-v
    # into the last layer's matmuls so ACT Square reads psi3 straight from PSUM.
    P = np.zeros((128, 128))
    P[0:64, 0:64] = np.eye(64)
    P[0:64, 64:128] = np.eye(64)
    P[64:128, 0:64] = np.eye(64)
    P[64:128, 64:128] = -np.eye(64)

    c_ev = np.zeros((4, 128, 8), np.float16)
    for sl in range(4):
        for g in range(2):
            c_ev[sl, g * 32:(g + 1) * 32, 2 * sl + g] = 0.5
            c_ev[sl, 64 + g * 32:64 + (g + 1) * 32, 2 * sl + g] = -0.5

    bld_all = [bld_T(T).T for T in range(4)]
    bc_all = [bc_T(T).T for T in range(4)]
    bs_all = [bs_T(T).T for T in range(4)]
    return dict(
        c_sig=np.ascontiguousarray(c_sig),
        c_bld=np.ascontiguousarray(np.stack(bld_all).astype(np.float16)),
        c_bc=np.ascontiguousarray(np.stack(bc_all).astype(np.float16)),
        c_bs=np.ascontiguousarray(np.stack(bs_all).astype(np.float16)),
        c_f=np.ascontiguousarray(
            np.stack([Fhat[1].T, (Fhat[1] @ SWAP).T,
                      (P @ Fhat[2]).T, (P @ Fhat[2] @ SWAP).T]).astype(np.float16)),
        c_ev=np.ascontiguousarray(c_ev),
    )


def _angle_blocks(pix):
    """pix [P, 12] f32 -> A [P/4096 pairs, 3, 34, 512] f32.
    Patch linear order ((m*2+tt)*2+g)*512+n; pair P holds m in {2P, 2P+1};
    p-tile group g_new = (m%2)*4 + tt*2 + g; row g_new*4+w = angle of wire w,
    row 32 = 1.0 (pi/2 offset selector)."""
    n_pair = pix.shape[0] // 4096
    px = pix.reshape(n_pair, 2, 2, 2, 512, 12)   # (pair, m', tt, g, n, 12)
    A = np.zeros((n_pair, 3, 34, 512), np.float32)
    for l in range(3):
        for mp in range(2):
            for tt in range(2):
                for g in range(2):
                    g_new = 4 * mp + 2 * tt + g
                    for w in range(4):
                        A[:, l, g_new * 4 + w, :] = px[:, mp, tt, g, :, 4 * l + w]
        A[:, l, 32, :] = 1.0
    return A


# ----------------------------------------------------------------------------
# device program
# ----------------------------------------------------------------------------

def _build_nc(n_tiles=TILES_PER_CORE, repeat=1):
    """n_tiles = old 1024-patch tiles per core; must be divisible by 4."""
    import contextlib
    import concourse.mybir as mybir
    from concourse import bacc
    from concourse.ap import AP
    from concourse.tile import TileContext

    F32 = mybir.dt.float32
    F32R = mybir.dt.float32r
    F16 = mybir.dt.float16
    AF = mybir.ActivationFunctionType

    assert n_tiles % 4 == 0
    n_pair = n_tiles // 4

    nc = bacc.Bacc(None, target_bir_lowering=False, debug=False)
    ang_d = nc.declare_dram_parameter("ang", [n_pair, 3, 34, 512], F32R,
                                      isOutput=False)
    csig_d = nc.declare_dram_parameter("c_sig", [34, 128], F32R, isOutput=False)
    cbld_d = nc.declare_dram_parameter("c_bld", [4, 128, 128], F16, isOutput=False)
    cbc_d = nc.declare_dram_parameter("c_bc", [4, 128, 128], F16, isOutput=False)
    cbs_d = nc.declare_dram_parameter("c_bs", [4, 128, 128], F16, isOutput=False)
    cf_d = nc.declare_dram_parameter("c_f", [4, 128, 128], F16, isOutput=False)
    cev_d = nc.declare_dram_parameter("c_ev", [4, 128, 8], F16, isOutput=False)
    ev_d = nc.declare_dram_parameter("ev", [n_pair, 8, 512], F32, isOutput=True)

    BA = int(os.environ.get("BUFS_ANG", "9"))
    BP = int(os.environ.get("BUFS_P", "5"))
    BW = int(os.environ.get("BUFS_WRK", "5"))
    BC = int(os.environ.get("BUFS_CS", "6"))
    BS = int(os.environ.get("BUFS_PSIS", "4"))
    BM = int(os.environ.get("BUFS_MMT", "6"))
    # how many of each 4 consecutive bc/bs staging copies run on DVE (rest ACT)
    NDVE_CS = int(os.environ.get("NDVE_CS", "2"))
    # of each mh-layer's 2 psi staging copies, how many run on DVE (rest ACT)
    NDVE_PSI = int(os.environ.get("NDVE_PSI", "0"))
    DO_WRAP = os.environ.get("NO_WRAP", "0") != "1"

    with TileContext(nc) as tc:
        with (
            tc.tile_pool(name="const", bufs=1) as cpool,
            tc.tile_pool(name="angp", bufs=BA) as angp,
            tc.tile_pool(name="wrk", bufs=BW) as wrk,
            tc.tile_pool(name="pp", bufs=BP) as pp,
            tc.tile_pool(name="csp", bufs=BC) as csp,
            tc.tile_pool(name="psis", bufs=BS) as psis,
            tc.tile_pool(name="mmt", bufs=BM) as mmt,
            tc.tile_pool(name="qp", bufs=int(os.environ.get("BUFS_Q", "3"))) as qp,
            tc.tile_pool(name="evs", bufs=3) as evs,
            tc.tile_pool(name="sigp", bufs=1, space="PSUM") as sigp,
            tc.tile_pool(name="bcbs", bufs=2, space="PSUM") as bcbs,
            tc.tile_pool(name="psip", bufs=2, space="PSUM") as psip,
            tc.tile_pool(name="evp", bufs=1, space="PSUM") as evp,
        ):
            c_sig = cpool.tile([34, 128], F32R)
            nc.sync.dma_start(out=c_sig[:], in_=csig_d[:])
            c_bld = []
            c_bc = []
            c_bs = []
            for T in range(4):
                tb = cpool.tile([128, 128], F16, tag=f"bld{T}")
                nc.sync.dma_start(out=tb[:], in_=cbld_d[T])
                c_bld.append(tb)
                t1 = cpool.tile([128, 128], F16, tag=f"bc{T}")
                nc.sync.dma_start(out=t1[:], in_=cbc_d[T])
                c_bc.append(t1)
                t2 = cpool.tile([128, 128], F16, tag=f"bs{T}")
                nc.sync.dma_start(out=t2[:], in_=cbs_d[T])
                c_bs.append(t2)
            c_f = []
            for k in range(4):
                tf = cpool.tile([128, 128], F16, tag=f"f{k}")
                nc.sync.dma_start(out=tf[:], in_=cf_d[k])
                c_f.append(tf)
            c_ev = []
            for sl in range(4):
                te = cpool.tile([128, 8], F16, tag=f"ev{sl}")
                nc.sync.dma_start(out=te[:], in_=cev_d[sl])
                c_ev.append(te)

            rep_ctx = (tc.For_i(0, repeat, 1) if repeat > 1
                       else contextlib.nullcontext())
            with rep_ctx:
                for Pp in range(n_pair):
                    a_ls = []
                    for l in range(3):
                        a_l = angp.tile([34, 512], F32R, tag=f"ang{l}")
                        nc.gpsimd.dma_start(out=a_l[:], in_=ang_d[Pp, l])
                        a_ls.append(a_l)

                    psi_s = [None, None]      # per mh, fp16 SBUF [128,1024]
                    q2 = [None, None]         # per mh, fp16 squares [128,1024]
                    cs_k = 0                  # bc/bs staging copy round-robin
                    for l in range(3):
                        sig = sigp.tile([128, 512], F32, tag="sig")
                        nc.tensor.matmul(sig[:], c_sig[:], a_ls[l][:],
                                         start=True, stop=True)
                        if DO_WRAP:
                            w = wrk.tile([128, 512], F32, tag="w")
                            nc.vector.add_range_wrap(
                                w[:], sig[:], shift=0.0, bound=PI, period=2 * PI)
                            p = pp.tile([128, 512], F16, tag="p")
                            nc.scalar.activation(p[:], w[:], AF.Sin)
                        else:
                            p = pp.tile([128, 512], F16, tag="p")
                            nc.scalar.activation(p[:], sig[:], AF.Sin)

                        for mh in range(2):
                            T0 = 2 * mh
                            psi_p = [psip.tile([128, 512], F32, tag="psi",
                                               name=f"psi{ti}")
                                     for ti in range(2)]
                            if l == 0:
                                for ti, T in enumerate((T0, T0 + 1)):
                                    nc.tensor.matmul(psi_p[ti][:],
                                                     c_bld[T][:], p[:],
                                                     start=True, stop=True)
                            else:
                                bcP = bcbs.tile([128, 1024], F32, tag="bcbs")
                                nc.tensor.matmul(bcP[:, 0:512], c_bc[T0][:],
                                                 p[:], start=True, stop=True)
                                nc.tensor.matmul(bcP[:, 512:1024],
                                                 c_bc[T0 + 1][:],
                                                 p[:], start=True, stop=True)
                                bsP = bcbs.tile([128, 1024], F32, tag="bcbs")
                                nc.tensor.matmul(bsP[:, 0:512], c_bs[T0][:],
                                                 p[:], start=True, stop=True)
                                nc.tensor.matmul(bsP[:, 512:1024],
                                                 c_bs[T0 + 1][:],
                                                 p[:], start=True, stop=True)
                                bc_s = csp.tile([128, 1024], F16, tag="cs")
                                bs_s = csp.tile([128, 1024], F16, tag="cs")
                                for src_t, dst in ((bcP, bc_s), (bsP, bs_s)):
                                    if cs_k % 4 < NDVE_CS:
                                        nc.vector.tensor_copy(dst[:], src_t[:])
                                    else:
                                        nc.scalar.copy(out=dst[:], in_=src_t[:])
                                    cs_k += 1
                                m1 = mmt.tile([128, 1024], F16, tag="m")
                                nc.vector.tensor_mul(m1[:], bc_s[:], psi_s[mh][:])
                                m2 = mmt.tile([128, 1024], F16, tag="m")
                                nc.vector.tensor_mul(m2[:], bs_s[:],
                                                     psi_s[mh][:])
                                base = 2 * (l - 1)
                                for ti in range(2):
                                    sl_c = slice(512 * ti, 512 * ti + 512)
                                    nc.tensor.matmul(psi_p[ti][:], c_f[base][:],
                                                     m1[:, sl_c], start=True,
                                                     stop=False)
                                    nc.tensor.matmul(psi_p[ti][:],
                                                     c_f[base + 1][:],
                                                     m2[:, sl_c], start=False,
                                                     stop=True)
                            if l < 2:
                                ps_t = psis.tile([128, 1024], F16, tag="psis")
                                for ti in range(2):
                                    if ti < NDVE_PSI:
                                        nc.vector.tensor_copy(
                                            ps_t[:, 512 * ti:512 * ti + 512],
                                            psi_p[ti][:])
                                    else:
                                        nc.scalar.copy(
                                            out=ps_t[:, 512 * ti:512 * ti + 512],
                                            in_=psi_p[ti][:])
                                psi_s[mh] = ps_t
                            else:
                                q_t = qp.tile([128, 1024], F16, tag="q2")
                                for ti in range(2):
                                    nc.scalar.activation(
                                        q_t[:, 512 * ti:512 * ti + 512],
                                        psi_p[ti][:], AF.Square)
                                q2[mh] = q_t

                    evt = evp.tile([8, 512], F32, tag="ev")
                    for mh in range(2):
                        for ti in range(2):
                            sl = 2 * mh + ti
                            nc.tensor.matmul(evt[:],
                                             c_ev[sl][:],
                                             q2[mh][:, 512 * ti:512 * ti + 512],
                                             start=(sl == 0), stop=(sl == 3))
                    ev_s = evs.tile([8, 512], F32, tag="evs")
                    nc.scalar.copy(out=ev_s[:], in_=evt[:])
                    nc.sync.dma_start(out=ev_d[Pp], in_=ev_s[:])

    nc.finalize()
    return nc


def _get_nc(repeat=_REPEAT):
    key = ("nc", repeat)
    if key not in _CACHE:
        _CACHE[key] = _build_nc(repeat=repeat)
    return _CACHE[key]


# ----------------------------------------------------------------------------
# entry point
# ----------------------------------------------------------------------------

def kernel(x, thetas, phis):
    from concourse.bass_utils import run_bass_kernel_spmd

    x = np.asarray(x, np.float32)
    thetas = np.asarray(thetas, np.float32)
    phis = np.asarray(phis, np.float32)
    B, C, H, W = x.shape
    H2, W2 = H // 2, W // 2
    pix = (x.reshape(B, 3, H2, 2, W2, 2)
             .transpose(0, 2, 4, 1, 3, 5)
             .reshape(B * H2 * W2, 12))

    A = _angle_blocks(pix)                    # [64 pairs, 3, 34, 512]
    consts = _build_constants(thetas, phis)
    per_core = A.shape[0] // N_CORES
    in_maps = [{"ang": np.ascontiguousarray(A[c * per_core:(c + 1) * per_core]),
                **consts} for c in range(N_CORES)]

    nc = _get_nc()
    res = run_bass_kernel_spmd(nc, in_maps, list(range(N_CORES)))
    # ev_d [n_pair, 8, 512]: row 2*sl+g of pair P -> old tile 4*P+sl
    evs = [res.results[c]["ev"].reshape(-1, 4, 2, 512).reshape(-1)
           for c in range(N_CORES)]
    ev = np.concatenate(evs)
    return ev.reshape(B, 1, H2, W2).astype(np.float32)
